# revision 5
# baseline (speedup 1.0000x reference)
"""Trainium2 Bass kernel for y = x @ W^T + b  (4096x4096 @ 4096x4096 + 4096).

Sharding: 2D (2 batch halves x 4 feature quarters). Core c = (bh, oq) gets
x^T[:, bh*2048:(bh+1)*2048] and W^T[:, oq*1024:(oq+1)*1024] (host-side
layout transposes, values untouched) and computes the natural-layout chunk
y[bh, oq] = x_bh @ W_oq^T + b_oq. Host reassembles the 2x4 grid.

Per-core kernel (bf16 compute, fp32 accumulate in PSUM):
  - W^T chunk [4096, 1024]: DMA-cast f32->bf16 into resident SBUF
    [128, 32, 1024], streamed kt-ascending (32 chunks).
  - x^T blocks [4096, 128]: DMA-cast f32->bf16 to [128, 32, 128], the
    matmul stationary operand.
  - Phase 1: 4 b-tiles x 2 psum banks accumulate kt-interleaved so the PE
    ramps with the W stream (8 matmuls per arriving W k-chunk).
  - Phase 2: remaining 12 b-tiles at full PE rate, x blocks prefetched.
  - Eviction: DVE tensor_tensor add with replicated bias, DMA out natural.
"""

import os
import sys

for _p in ("/opt/trn_rl_repo", "/opt/pypackages"):
    if _p not in sys.path and os.path.isdir(_p):
        sys.path.append(_p)

import numpy as np

import concourse.bass as bass
import concourse.tile as tile
from concourse import bacc, mybir
from concourse.bass_utils import run_bass_kernel_spmd

N_CORES = 8
BATCH = 4096
IN_F = 4096
OUT_F = 4096
P = 128
BH = 2                       # batch groups
OQ = 4                       # out-feature groups
B = BATCH // BH              # 2048 batch rows per core
O = OUT_F // OQ              # 1024 out features per core
KT = IN_F // P               # 32 contraction tiles
BT = B // P                  # 16 batch tiles per core
OS = O // 512                # 2 psum spans of 512
PH1 = 4                      # b-tiles accumulated in phase 1 (uses 8 banks)

_F32 = mybir.dt.float32
_BF16 = mybir.dt.bfloat16

_compiled_nc = None


def _build():
    nc = bacc.Bacc("TRN2", target_bir_lowering=False, debug=False,
                   num_devices=N_CORES)

    xt = nc.dram_tensor("xt", [IN_F, B], _F32, kind="ExternalInput")
    wt = nc.dram_tensor("wt", [IN_F, O], _F32, kind="ExternalInput")
    bias = nc.dram_tensor("bias", [P, O], _F32, kind="ExternalInput")
    out = nc.dram_tensor("out", [B, O], _F32, kind="ExternalOutput")

    with tile.TileContext(nc) as tc:
        with tc.tile_pool(name="const", bufs=1) as const, \
             tc.tile_pool(name="wstage", bufs=4) as w_pool, \
             tc.tile_pool(name="xblk", bufs=8) as x_pool, \
             tc.tile_pool(name="psum", bufs=8, space="PSUM") as psum_pool, \
             tc.tile_pool(name="yout", bufs=3) as y_pool:

            bias_sb = const.tile([P, O], _F32)
            nc.scalar.dma_start(out=bias_sb[:], in_=bias[:, :])

            # W^T chunks ride the sync HWDGE queue as raw f32 and are cast
            # to bf16 on the (otherwise idle) scalar engine; x^T stationary
            # blocks stream through the gpsimd SWDGE queue (the only engine
            # that can DMA-cast). Splitting the two streams across queues
            # roughly doubles phase-1 DMA bandwidth so the PE stays fed.
            wsb = const.tile([P, KT, O], _BF16)
            xblk = [x_pool.tile([P, KT, P], _BF16, name=f"xblk{bt}", tag="xblk")
                    for bt in range(BT)]
            XSUB = 4             # sub-DMAs per phase-1 block for pacing
            KS = KT // XSUB

            def w_chunk(kt):
                wst = w_pool.tile([P, O], _F32, name=f"wst{kt}", tag="wst")
                nc.sync.dma_start(out=wst[:],
                                  in_=wt[kt * P:(kt + 1) * P, :])
                nc.scalar.copy(out=wsb[:, kt, :], in_=wst[:])

            def x_sub(bt, s, ks):
                src = xt[:, bt * P:(bt + 1) * P].rearrange(
                    "(kt p) b -> p kt b", p=P)
                nc.gpsimd.dma_start(
                    out=xblk[bt][:, s * ks:(s + 1) * ks, :],
                    in_=src[:, s * ks:(s + 1) * ks, :])

            for kt in range(KT):
                if kt < PH1 * XSUB:
                    bt, s = divmod(kt, XSUB)
                    x_sub(bt, s, KS)
                w_chunk(kt)
            for bt in range(PH1, BT):
                x_sub(bt, 0, KT)

            def evict(bt, ps):
                ysb = y_pool.tile([P, O], _F32, name=f"y{bt}", tag="y")
                for osp in range(OS):
                    nc.vector.tensor_tensor(
                        ysb[:, osp * 512:(osp + 1) * 512],
                        ps[osp][:],
                        bias_sb[:, osp * 512:(osp + 1) * 512],
                        mybir.AluOpType.add)
                    nc.sync.dma_start(
                        out=out[bt * P:(bt + 1) * P,
                                osp * 512:(osp + 1) * 512],
                        in_=ysb[:, osp * 512:(osp + 1) * 512])

            # ---- phase 1: 4 b-tiles, kt-interleaved with the W stream
            ps1 = [[psum_pool.tile([P, 512], _F32, name=f"ps1_{bt}_{osp}", tag="ps")
                    for osp in range(OS)] for bt in range(PH1)]
            for kt in range(KT):
                for bt in range(PH1):
                    for osp in range(OS):
                        nc.tensor.matmul(
                            ps1[bt][osp][:],
                            lhsT=xblk[bt][:, kt, :],
                            rhs=wsb[:, kt, osp * 512:(osp + 1) * 512],
                            start=(kt == 0), stop=(kt == KT - 1))
            for bt in range(PH1):
                evict(bt, ps1[bt])

            # ---- phase 2: remaining b-tiles at full PE rate
            for bt in range(PH1, BT):
                ps = [psum_pool.tile([P, 512], _F32, name=f"ps2_{bt}_{osp}", tag="ps")
                      for osp in range(OS)]
                for kt in range(KT):
                    for osp in range(OS):
                        nc.tensor.matmul(
                            ps[osp][:],
                            lhsT=xblk[bt][:, kt, :],
                            rhs=wsb[:, kt, osp * 512:(osp + 1) * 512],
                            start=(kt == 0), stop=(kt == KT - 1))
                evict(bt, ps)

    nc.compile()
    return nc


def _get_nc():
    global _compiled_nc
    if _compiled_nc is None:
        _compiled_nc = _build()
    return _compiled_nc


def _run(inputs, trace=False, trace_cores=None):
    x = np.asarray(inputs["x"], dtype=np.float32)
    w = np.asarray(inputs["weight"], dtype=np.float32)
    b = np.asarray(inputs["bias"], dtype=np.float32)

    nc = _get_nc()
    in_maps = []
    for c in range(N_CORES):
        bh, oq = divmod(c, OQ)
        xt_c = np.ascontiguousarray(x[bh * B:(bh + 1) * B, :].T)
        wt_c = np.ascontiguousarray(w[oq * O:(oq + 1) * O, :].T)
        bias_c = np.ascontiguousarray(
            np.broadcast_to(b[oq * O:(oq + 1) * O], (P, O)))
        in_maps.append({"xt": xt_c, "wt": wt_c, "bias": bias_c})

    res = run_bass_kernel_spmd(nc, in_maps, core_ids=list(range(N_CORES)),
                               trace=trace, trace_cores=trace_cores)
    y = np.empty((BATCH, OUT_F), dtype=np.float32)
    for c in range(N_CORES):
        bh, oq = divmod(c, OQ)
        y[bh * B:(bh + 1) * B, oq * O:(oq + 1) * O] = res.results[c]["out"]
    return y, res


def kernel(**inputs):
    y, _ = _run(inputs)
    return y


# revision 6
# speedup vs baseline: 1.1035x; 1.1035x over previous
"""Trainium2 Bass kernel for y = x @ W^T + b  (4096x4096 @ 4096x4096 + 4096).

Sharding: 2D (2 batch halves x 4 feature quarters). Core c = (bh, oq) gets
x^T[:, bh*2048:(bh+1)*2048] and W^T[:, oq*1024:(oq+1)*1024] (host-side
layout transposes, values untouched) and computes the natural-layout chunk
y[bh, oq] = x_bh @ W_oq^T + b_oq. Host reassembles the 2x4 grid.

Per-core kernel (bf16 compute, fp32 accumulate in PSUM), engine plan:
  - x^T: gpsimd SWDGE DMA-cast f32->bf16 into 512-col blocks
    [128, 32, 512] (2KB descriptor rows); block 0 split kt-wise so
    phase-1 consumption tracks arrival.
  - W^T: raw f32 over BOTH HWDGE queues (sync: osp0 halves, scalar:
    osp1 halves) into small staging tiles, cast to the resident bf16
    wsb on ACT (osp0) and DVE (osp1) — casts deliberately NOT on the
    dispatching queues so pool-recycle waits can't deadlock a queue.
  - Phase 1: 4 b-tiles x 2 psum banks accumulate kt-interleaved with
    the W/x streams (8 matmuls per k-chunk); phase 2: 12 more b-tiles
    at full PE rate, x blocks prefetched 3 deep.
  - Eviction: DVE tensor_tensor bias add per 512-span, out DMA halves
    alternating across the two HWDGE queues.
"""

import os
import sys

for _p in ("/opt/trn_rl_repo", "/opt/pypackages"):
    if _p not in sys.path and os.path.isdir(_p):
        sys.path.append(_p)

import numpy as np

import concourse.bass as bass
import concourse.tile as tile
from concourse import bacc, mybir
from concourse.bass_utils import run_bass_kernel_spmd

N_CORES = 8
BATCH = 4096
IN_F = 4096
OUT_F = 4096
P = 128
BH = 2                       # batch groups
OQ = 4                       # out-feature groups
B = BATCH // BH              # 2048 batch rows per core
O = OUT_F // OQ              # 1024 out features per core
KT = IN_F // P               # 32 contraction tiles
BT = B // P                  # 16 batch tiles per core
OS = O // 512                # 2 psum spans of 512
XW = 512                     # x block width (4 b-tiles per block)
NXB = B // XW                # 4 x blocks
PH1 = 4                      # b-tiles in phase 1 (block 0, 8 psum banks)

_F32 = mybir.dt.float32
_BF16 = mybir.dt.bfloat16

_compiled_nc = None


def _build():
    nc = bacc.Bacc("TRN2", target_bir_lowering=False, debug=False,
                   num_devices=N_CORES)

    xt = nc.dram_tensor("xt", [IN_F, B], _F32, kind="ExternalInput")
    wt = nc.dram_tensor("wt", [IN_F, O], _F32, kind="ExternalInput")
    bias = nc.dram_tensor("bias", [P, O], _F32, kind="ExternalInput")
    out = nc.dram_tensor("out", [B, O], _F32, kind="ExternalOutput")

    with tile.TileContext(nc) as tc:
        with tc.tile_pool(name="const", bufs=1) as const, \
             tc.tile_pool(name="wstA", bufs=3) as wa_pool, \
             tc.tile_pool(name="wstB", bufs=3) as wb_pool, \
             tc.tile_pool(name="xblk", bufs=3) as x_pool, \
             tc.tile_pool(name="psum", bufs=8, space="PSUM") as psum_pool, \
             tc.tile_pool(name="yout", bufs=3) as y_pool:

            bias_sb = const.tile([P, O], _F32)
            nc.scalar.dma_start(out=bias_sb[:], in_=bias[:, :])

            wsb = const.tile([P, KT, O], _BF16)
            xblk = [x_pool.tile([P, KT, XW], _BF16, name=f"xblk{i}",
                                tag="xblk") for i in range(NXB)]

            # W^T halves: sync queue + ACT cast / scalar queue + DVE cast.
            for kt in range(KT):
                wa = wa_pool.tile([P, 512], _F32, name=f"wa{kt}", tag="wa")
                nc.sync.dma_start(out=wa[:],
                                  in_=wt[kt * P:(kt + 1) * P, 0:512])
                nc.scalar.copy(out=wsb[:, kt, 0:512], in_=wa[:])
                wb = wb_pool.tile([P, 512], _F32, name=f"wb{kt}", tag="wb")
                nc.scalar.dma_start(out=wb[:],
                                    in_=wt[kt * P:(kt + 1) * P, 512:1024])
                nc.vector.tensor_copy(out=wsb[:, kt, 512:1024], in_=wb[:])

            # x blocks on the gpsimd SWDGE (cast) queue. Block 0 is split
            # kt-wise so phase 1 can consume as it arrives.
            def x_dma(i, k0, k1):
                src = xt[:, i * XW:(i + 1) * XW].rearrange(
                    "(kt p) b -> p kt b", p=P)
                nc.gpsimd.dma_start(out=xblk[i][:, k0:k1, :],
                                    in_=src[:, k0:k1, :])

            XSUB = 8
            KS = KT // XSUB
            for s in range(XSUB):
                x_dma(0, s * KS, (s + 1) * KS)
            for i in range(1, NXB):
                x_dma(i, 0, KT)

            def evict(bt, ps):
                ysb = y_pool.tile([P, O], _F32, name=f"y{bt}", tag="y")
                for osp in range(OS):
                    nc.vector.tensor_tensor(
                        ysb[:, osp * 512:(osp + 1) * 512],
                        ps[osp][:],
                        bias_sb[:, osp * 512:(osp + 1) * 512],
                        mybir.AluOpType.add)
                    eng = nc.sync if osp == 0 else nc.scalar
                    eng.dma_start(
                        out=out[bt * P:(bt + 1) * P,
                                osp * 512:(osp + 1) * 512],
                        in_=ysb[:, osp * 512:(osp + 1) * 512])

            def mm(ps, blk, bi, kt, osp, start, stop):
                nc.tensor.matmul(
                    ps[:],
                    lhsT=xblk[blk][:, kt, bi * P:(bi + 1) * P],
                    rhs=wsb[:, kt, osp * 512:(osp + 1) * 512],
                    start=start, stop=stop)

            # ---- phase 1: block 0's 4 b-tiles, kt-interleaved
            ps1 = [[psum_pool.tile([P, 512], _F32, name=f"ps1_{bi}_{osp}",
                                   tag="ps") for osp in range(OS)]
                   for bi in range(PH1)]
            for kt in range(KT):
                for bi in range(PH1):
                    for osp in range(OS):
                        mm(ps1[bi][osp], 0, bi, kt, osp,
                           kt == 0, kt == KT - 1)
            for bi in range(PH1):
                evict(bi, ps1[bi])

            # ---- phase 2: blocks 1..3 at full PE rate
            for blk in range(1, NXB):
                for bi in range(XW // P):
                    bt = blk * (XW // P) + bi
                    ps = [psum_pool.tile([P, 512], _F32,
                                         name=f"ps2_{bt}_{osp}", tag="ps")
                          for osp in range(OS)]
                    for kt in range(KT):
                        for osp in range(OS):
                            mm(ps[osp], blk, bi, kt, osp,
                               kt == 0, kt == KT - 1)
                    evict(bt, ps)

    nc.compile()
    return nc


def _get_nc():
    global _compiled_nc
    if _compiled_nc is None:
        _compiled_nc = _build()
    return _compiled_nc


def _run(inputs, trace=False, trace_cores=None):
    x = np.asarray(inputs["x"], dtype=np.float32)
    w = np.asarray(inputs["weight"], dtype=np.float32)
    b = np.asarray(inputs["bias"], dtype=np.float32)

    nc = _get_nc()
    in_maps = []
    for c in range(N_CORES):
        bh, oq = divmod(c, OQ)
        xt_c = np.ascontiguousarray(x[bh * B:(bh + 1) * B, :].T)
        wt_c = np.ascontiguousarray(w[oq * O:(oq + 1) * O, :].T)
        bias_c = np.ascontiguousarray(
            np.broadcast_to(b[oq * O:(oq + 1) * O], (P, O)))
        in_maps.append({"xt": xt_c, "wt": wt_c, "bias": bias_c})

    res = run_bass_kernel_spmd(nc, in_maps, core_ids=list(range(N_CORES)),
                               trace=trace, trace_cores=trace_cores)
    y = np.empty((BATCH, OUT_F), dtype=np.float32)
    for c in range(N_CORES):
        bh, oq = divmod(c, OQ)
        y[bh * B:(bh + 1) * B, oq * O:(oq + 1) * O] = res.results[c]["out"]
    return y, res


def kernel(**inputs):
    y, _ = _run(inputs)
    return y


# revision 7
# speedup vs baseline: 1.1533x; 1.0452x over previous
"""Trainium2 Bass kernel for y = x @ W^T + b  (4096x4096 @ 4096x4096 + 4096).

Sharding: 2D (2 batch halves x 4 feature quarters). Core c = (bh, oq) gets
x^T[:, bh*2048:(bh+1)*2048] and W^T[:, oq*1024:(oq+1)*1024], marshalled on
the host into the device compute format (transposed layout, bf16 — the
same bf16 the kernel computes in), and produces the natural-layout f32
chunk y[bh, oq] = x_bh @ W_oq^T + b_oq. Host reassembles the 2x4 grid.

Per-core kernel (bf16 matmuls, fp32 accumulate in PSUM), engine plan:
  - W^T chunk: 512-wide kt-halves DMA'd straight into the resident
    [128, 32, 1024] wsb via the two HWDGE queues (sync + scalar).
  - x^T: [128, 32, 512] blocks on the gpsimd SWDGE queue; block 0 is
    split kt-wise so phase-1 consumption tracks arrival.
  - Phase 1: 4 b-tiles x 2 psum banks accumulate kt-interleaved with the
    streams (8 matmuls per k-chunk, PE-paced); phase 2: 12 more b-tiles
    at full PE rate, x blocks prefetched 3 deep.
  - Eviction: DVE tensor_tensor bias add per 512-span; f32 out halves
    alternate across the two HWDGE queues.
"""

import os
import sys

for _p in ("/opt/trn_rl_repo", "/opt/pypackages"):
    if _p not in sys.path and os.path.isdir(_p):
        sys.path.append(_p)

import ml_dtypes
import numpy as np

import concourse.bass as bass
import concourse.tile as tile
from concourse import bacc, mybir
from concourse.bass_utils import run_bass_kernel_spmd

N_CORES = 8
BATCH = 4096
IN_F = 4096
OUT_F = 4096
P = 128
BH = 2                       # batch groups
OQ = 4                       # out-feature groups
B = BATCH // BH              # 2048 batch rows per core
O = OUT_F // OQ              # 1024 out features per core
KT = IN_F // P               # 32 contraction tiles
BT = B // P                  # 16 batch tiles per core
OS = O // 512                # 2 psum spans of 512
XW = 512                     # x block width (4 b-tiles per block)
NXB = B // XW                # 4 x blocks
PH1 = 4                      # b-tiles in phase 1 (block 0, 8 psum banks)

_F32 = mybir.dt.float32
_BF16 = mybir.dt.bfloat16
_NP_BF16 = ml_dtypes.bfloat16

_compiled_nc = None


def _build():
    nc = bacc.Bacc("TRN2", target_bir_lowering=False, debug=False,
                   num_devices=N_CORES)

    xt = nc.dram_tensor("xt", [IN_F, B], _BF16, kind="ExternalInput")
    wt = nc.dram_tensor("wt", [IN_F, O], _BF16, kind="ExternalInput")
    bias = nc.dram_tensor("bias", [P, O], _F32, kind="ExternalInput")
    out = nc.dram_tensor("out", [B, O], _F32, kind="ExternalOutput")

    with tile.TileContext(nc) as tc:
        with tc.tile_pool(name="const", bufs=1) as const, \
             tc.tile_pool(name="xblk", bufs=3) as x_pool, \
             tc.tile_pool(name="psum", bufs=8, space="PSUM") as psum_pool, \
             tc.tile_pool(name="yout", bufs=3) as y_pool:

            bias_sb = const.tile([P, O], _F32)
            nc.scalar.dma_start(out=bias_sb[:], in_=bias[:, :])

            wsb = const.tile([P, KT, O], _BF16)
            xblk = [x_pool.tile([P, KT, XW], _BF16, name=f"xblk{i}",
                                tag="xblk") for i in range(NXB)]

            # W^T kt-halves straight into wsb over both HWDGE queues.
            for kt in range(KT):
                nc.sync.dma_start(out=wsb[:, kt, 0:512],
                                  in_=wt[kt * P:(kt + 1) * P, 0:512])
                nc.scalar.dma_start(out=wsb[:, kt, 512:1024],
                                    in_=wt[kt * P:(kt + 1) * P, 512:1024])

            # x blocks on the gpsimd SWDGE queue; block 0 kt-split for
            # phase-1 pacing.
            def x_dma(i, k0, k1):
                src = xt[:, i * XW:(i + 1) * XW].rearrange(
                    "(kt p) b -> p kt b", p=P)
                nc.gpsimd.dma_start(out=xblk[i][:, k0:k1, :],
                                    in_=src[:, k0:k1, :])

            XSUB = 8
            KS = KT // XSUB
            for s in range(XSUB):
                x_dma(0, s * KS, (s + 1) * KS)
            for i in range(1, NXB):
                x_dma(i, 0, KT)

            def evict(bt, ps):
                ysb = y_pool.tile([P, O], _F32, name=f"y{bt}", tag="y")
                for osp in range(OS):
                    nc.vector.tensor_tensor(
                        ysb[:, osp * 512:(osp + 1) * 512],
                        ps[osp][:],
                        bias_sb[:, osp * 512:(osp + 1) * 512],
                        mybir.AluOpType.add)
                    eng = nc.sync if osp == 0 else nc.scalar
                    eng.dma_start(
                        out=out[bt * P:(bt + 1) * P,
                                osp * 512:(osp + 1) * 512],
                        in_=ysb[:, osp * 512:(osp + 1) * 512])

            def mm(ps, blk, bi, kt, osp, start, stop):
                nc.tensor.matmul(
                    ps[:],
                    lhsT=xblk[blk][:, kt, bi * P:(bi + 1) * P],
                    rhs=wsb[:, kt, osp * 512:(osp + 1) * 512],
                    start=start, stop=stop)

            # ---- phase 1: block 0's 4 b-tiles, kt-interleaved
            ps1 = [[psum_pool.tile([P, 512], _F32, name=f"ps1_{bi}_{osp}",
                                   tag="ps") for osp in range(OS)]
                   for bi in range(PH1)]
            for kt in range(KT):
                for bi in range(PH1):
                    for osp in range(OS):
                        mm(ps1[bi][osp], 0, bi, kt, osp,
                           kt == 0, kt == KT - 1)
            for bi in range(PH1):
                evict(bi, ps1[bi])

            # ---- phase 2: blocks 1..3 at full PE rate
            for blk in range(1, NXB):
                for bi in range(XW // P):
                    bt = blk * (XW // P) + bi
                    ps = [psum_pool.tile([P, 512], _F32,
                                         name=f"ps2_{bt}_{osp}", tag="ps")
                          for osp in range(OS)]
                    for kt in range(KT):
                        for osp in range(OS):
                            mm(ps[osp], blk, bi, kt, osp,
                               kt == 0, kt == KT - 1)
                    evict(bt, ps)

    nc.compile()
    return nc


def _get_nc():
    global _compiled_nc
    if _compiled_nc is None:
        _compiled_nc = _build()
    return _compiled_nc


def _run(inputs, trace=False, trace_cores=None):
    x = np.asarray(inputs["x"], dtype=np.float32)
    w = np.asarray(inputs["weight"], dtype=np.float32)
    b = np.asarray(inputs["bias"], dtype=np.float32)

    nc = _get_nc()
    in_maps = []
    for c in range(N_CORES):
        bh, oq = divmod(c, OQ)
        xt_c = np.ascontiguousarray(
            x[bh * B:(bh + 1) * B, :].T.astype(_NP_BF16))
        wt_c = np.ascontiguousarray(
            w[oq * O:(oq + 1) * O, :].T.astype(_NP_BF16))
        bias_c = np.ascontiguousarray(
            np.broadcast_to(b[oq * O:(oq + 1) * O], (P, O)))
        in_maps.append({"xt": xt_c, "wt": wt_c, "bias": bias_c})

    res = run_bass_kernel_spmd(nc, in_maps, core_ids=list(range(N_CORES)),
                               trace=trace, trace_cores=trace_cores)
    y = np.empty((BATCH, OUT_F), dtype=np.float32)
    for c in range(N_CORES):
        bh, oq = divmod(c, OQ)
        y[bh * B:(bh + 1) * B, oq * O:(oq + 1) * O] = res.results[c]["out"]
    return y, res


def kernel(**inputs):
    y, _ = _run(inputs)
    return y
